# revision 1
# baseline (speedup 1.0000x reference)
"""CPMAnt transformer block on 8 TRN2 NeuronCores.

Sharding (Megatron-style): core c owns 4 attention heads (q/k/v/o slices) and
1280 FFN columns (w0/w1/w_out slices). Activations are kept feature-major
(D on partitions) on device. Cross-core comms: 4 chunked AllReduces of the
attention output (feeds the FFN everywhere) and 4 chunked ReduceScatters of
the combined (attention + FFN) partials (feeds each core's 512-row slice of
the final residual). Host folds RMSNorm weights / softmax scale / mask into
the weights and position bias, transposes activations, and concatenates the
8 per-core (512, 2048) outputs.
"""

import math

import numpy as np

S = 2048
D = 4096
H = 32
DH = 128
FF = 10240
NCORES = 8
P = 128
HPC = H // NCORES            # 4 heads per core
WPC = HPC * DH               # 512   per-core qkv width
FPC = FF // NCORES           # 1280  per-core ff width
FCC = FPC // P               # 10
DC = D // P                  # 32
SCN = 4                      # S chunks
SCW = S // SCN               # 512
KC = S // P                  # 16 key chunks
EPS = 1e-6

_CACHE = {}


def _build(stages="all"):
    import concourse.bacc as bacc
    import concourse.tile as tile
    from concourse import mybir

    f32 = mybir.dt.float32
    bf = mybir.dt.bfloat16
    AF = mybir.ActivationFunctionType
    ALU = mybir.AluOpType
    AX = mybir.AxisListType
    RG = [list(range(NCORES))]

    nc = bacc.Bacc(None, num_devices=NCORES)

    hT = nc.dram_tensor("hT", [D, S], f32, kind="ExternalInput")
    h_own = nc.dram_tensor("h_own", [WPC, S], f32, kind="ExternalInput")
    wq = nc.dram_tensor("wq", [D, WPC], bf, kind="ExternalInput")
    wk = nc.dram_tensor("wk", [D, WPC], bf, kind="ExternalInput")
    wv = nc.dram_tensor("wv", [D, WPC], bf, kind="ExternalInput")
    wo = nc.dram_tensor("wo", [4, HPC, P, 8 * P], bf, kind="ExternalInput")
    pb = nc.dram_tensor("pb", [HPC, KC, SCN, P, SCW], bf, kind="ExternalInput")
    w0 = nc.dram_tensor("w0", [FCC, P, DC, P], bf, kind="ExternalInput")
    w1 = nc.dram_tensor("w1", [FCC, P, DC, P], bf, kind="ExternalInput")
    wout = nc.dram_tensor("wout", [8, FCC, P, 4 * P], bf, kind="ExternalInput")
    eye = nc.dram_tensor("eye", [P, P], bf, kind="ExternalInput")
    ones = nc.dram_tensor("ones", [P, P], bf, kind="ExternalInput")
    out = nc.dram_tensor("out", [WPC, S], f32, kind="ExternalOutput")

    from contextlib import ExitStack

    with tile.TileContext(nc) as tc:
        with ExitStack() as ctx:
            ep = ctx.enter_context
            dram = ep(tc.tile_pool(name="dram", bufs=1, space="DRAM"))
            singles = ep(tc.tile_pool(name="singles", bufs=1))
            xarena = ep(tc.tile_pool(name="xarena", bufs=1))
            hpool = ep(tc.tile_pool(name="hstr", bufs=3))
            sqpool = ep(tc.tile_pool(name="sqp", bufs=3))
            rpool = ep(tc.tile_pool(name="rbc", bufs=2))
            wpool = ep(tc.tile_pool(name="wstr", bufs=3))
            cpool = ep(tc.tile_pool(name="cpy", bufs=4))
            apool = ep(tc.tile_pool(name="att", bufs=2))
            ppool = ep(tc.tile_pool(name="pexp", bufs=2))
            spool = ep(tc.tile_pool(name="tiny", bufs=4))
            bpool = ep(tc.tile_pool(name="big16", bufs=2))
            atpool = ep(tc.tile_pool(name="atn", bufs=2))
            w01pool = ep(tc.tile_pool(name="w01", bufs=1))
            woutpool = ep(tc.tile_pool(name="wou", bufs=3))
            outpool = ep(tc.tile_pool(name="outp", bufs=2))
            psum = ep(tc.tile_pool(name="ps", bufs=4, space="PSUM"))
            # ---- DRAM scratch ----
            qT_d = dram.tile([HPC, SCN, P, SCW], bf, tag="qt")
            kT_d = dram.tile([HPC, P, S], bf, tag="kt")
            v_d = dram.tile([HPC, KC, P, DH], bf, tag="vd")
            arin = [dram.tile([D, SCW], bf, tag=f"arin{j}", name=f"arin{j}") for j in range(SCN)]
            arout = [
                dram.tile([D, SCW], bf, tag=f"arout{j}", name=f"arout{j}",
                          addr_space="Shared")
                for j in range(SCN)
            ]
            rsin = [dram.tile([D, SCW], bf, tag=f"rsin{j}", name=f"rsin{j}") for j in range(SCN)]
            rsout = [dram.tile([WPC, SCW], bf, tag=f"rsout{j}", name=f"rsout{j}") for j in range(SCN)]

            eye_sb = singles.tile([P, P], bf)
            nc.sync.dma_start(out=eye_sb[:], in_=eye[:])
            ones_sb = singles.tile([P, P], bf)
            nc.sync.dma_start(out=ones_sb[:], in_=ones[:])
            eps_t = singles.tile([P, 1], f32)
            nc.vector.memset(eps_t[:], EPS)

            # ================= stage 1+2: rmsnorm1 + QKV, per S-chunk =========
            for j in range(SCN):
                xj = xarena.tile([P, DC, SCW], bf, tag="acts")
                ss = psum.tile([P, SCW], f32, tag="psA")
                for d in range(DC):
                    hld = hpool.tile([P, SCW], f32)
                    nc.sync.dma_start(
                        out=hld[:], in_=hT[d * P:(d + 1) * P, j * SCW:(j + 1) * SCW]
                    )
                    nc.vector.tensor_copy(out=xj[:, d, :], in_=hld[:])
                    sq = sqpool.tile([P, SCW], bf)
                    nc.vector.tensor_mul(sq[:], hld[:], hld[:])
                    nc.tensor.matmul(
                        ss[:], lhsT=ones_sb[:], rhs=sq[:],
                        start=(d == 0), stop=(d == DC - 1),
                    )
                rbc = rpool.tile([P, SCW], f32)
                nc.scalar.activation(
                    out=rbc[:], in_=ss[:], func=AF.Sqrt, bias=eps_t[:], scale=1.0 / D
                )
                nc.vector.reciprocal(out=rbc[:], in_=rbc[:])
                for d in range(DC):
                    nc.vector.tensor_mul(xj[:, d, :], xj[:, d, :], rbc[:])

                # ---- Q^T then K^T: 4 psum banks each, accumulate over d ----
                for name, wsrc, dst in (("q", wq, qT_d), ("k", wk, kT_d)):
                    psb = [psum.tile([P, SCW], f32, tag="psA", name=f"ps_{name}{h2}") for h2 in range(HPC)]
                    for d in range(DC):
                        wd = wpool.tile([P, WPC], bf)
                        nc.sync.dma_start(out=wd[:], in_=wsrc[d * P:(d + 1) * P, :])
                        for h in range(HPC):
                            nc.tensor.matmul(
                                psb[h][:], lhsT=wd[:, h * DH:(h + 1) * DH],
                                rhs=xj[:, d, :], start=(d == 0), stop=(d == DC - 1),
                            )
                    for h in range(HPC):
                        cp = cpool.tile([P, SCW], bf)
                        nc.vector.tensor_copy(out=cp[:], in_=psb[h][:])
                        if dst is qT_d:
                            nc.sync.dma_start(out=dst[h, j], in_=cp[:])
                        else:
                            nc.sync.dma_start(
                                out=dst[h, :, j * SCW:(j + 1) * SCW], in_=cp[:]
                            )

                # ---- V (natural layout): 4 psum banks over d ----
                psv = [psum.tile([P, WPC], f32, tag="psA", name=f"psv{sl2}") for sl2 in range(SCW // P)]
                for d in range(DC):
                    wvd = wpool.tile([P, WPC], bf)
                    nc.sync.dma_start(out=wvd[:], in_=wv[d * P:(d + 1) * P, :])
                    for sl in range(SCW // P):
                        nc.tensor.matmul(
                            psv[sl][:], lhsT=xj[:, d, sl * P:(sl + 1) * P],
                            rhs=wvd[:], start=(d == 0), stop=(d == DC - 1),
                        )
                for sl in range(SCW // P):
                    cp = cpool.tile([P, WPC], bf)
                    nc.vector.tensor_copy(out=cp[:], in_=psv[sl][:])
                    for h in range(HPC):
                        nc.sync.dma_start(
                            out=v_d[h, j * (SCW // P) + sl],
                            in_=cp[:, h * DH:(h + 1) * DH],
                        )

            # ================= stage 3+4: attention + wo + AllReduce ==========
            for qg in (range(SCN) if stages in ("all", "qkv+attn", "noffn") else []):
                attnT = atpool.tile([P, HPC, SCW], bf)
                for h in range(HPC):
                    qt_h = apool.tile([P, SCW], bf, tag="qt")
                    nc.sync.dma_start(out=qt_h[:], in_=qT_d[h, qg])
                    kt_h = apool.tile([P, S], bf, tag="kt")
                    nc.sync.dma_start(out=kt_h[:], in_=kT_d[h])
                    v_h = apool.tile([P, KC, DH], bf, tag="vh")
                    nc.sync.dma_start(
                        out=v_h[:], in_=v_d[h].rearrange("kc p f -> p kc f")
                    )
                    pt_sb = bpool.tile([P, KC, SCW], bf, tag="big")
                    for qc in range(SCW // P):
                        pe = ppool.tile([P, S], bf)
                        sums = spool.tile([P, 4], f32, tag="sums")
                        for k4 in range(SCN):
                            pss = psum.tile([P, SCW], f32, tag="psA")
                            nc.tensor.matmul(
                                pss[:], lhsT=qt_h[:, qc * P:(qc + 1) * P],
                                rhs=kt_h[:, k4 * SCW:(k4 + 1) * SCW],
                                start=True, stop=True,
                            )
                            pbt = cpool.tile([P, SCW], bf, tag="pb", bufs=4)
                            nc.sync.dma_start(
                                out=pbt[:], in_=pb[h, qg * (SCW // P) + qc, k4]
                            )
                            sadd = sqpool.tile([P, SCW], f32, tag="sadd")
                            nc.vector.tensor_add(sadd[:], pss[:], pbt[:])
                            nc.scalar.activation(
                                out=pe[:, k4 * SCW:(k4 + 1) * SCW], in_=sadd[:],
                                func=AF.Exp, accum_out=sums[:, k4:k4 + 1],
                            )
                        stot = spool.tile([P, 1], f32, tag="stot")
                        nc.vector.tensor_reduce(
                            stot[:], sums[:], axis=AX.X, op=ALU.add
                        )
                        rs = spool.tile([P, 1], f32, tag="rs")
                        nc.vector.reciprocal(out=rs[:], in_=stot[:])
                        diag = spool.tile([P, P], bf, tag="diag")
                        nc.vector.tensor_scalar_mul(diag[:], eye_sb[:], rs[:])
                        # transpose+normalize: PT[k, q] = P[q, k] / s_q
                        pspt = psum.tile([P, KC, P], f32, tag="pt4", bufs=1)
                        for kc in range(KC):
                            nc.tensor.matmul(
                                pspt[:, kc, :], lhsT=pe[:, kc * P:(kc + 1) * P],
                                rhs=diag[:], start=True, stop=True,
                            )
                        nc.vector.tensor_copy(
                            out=pt_sb[:, :, qc * P:(qc + 1) * P], in_=pspt[:]
                        )
                    psav = psum.tile([P, SCW], f32, tag="psA")
                    for kc in range(KC):
                        nc.tensor.matmul(
                            psav[:], lhsT=v_h[:, kc, :], rhs=pt_sb[:, kc, :],
                            start=(kc == 0), stop=(kc == KC - 1),
                        )
                    nc.vector.tensor_copy(out=attnT[:, h, :], in_=psav[:])

                # ---- wo partials for this S chunk ----
                for dg in (range(4) if stages in ("all", "noffn") else []):
                    wo_sbs = []
                    for h in range(HPC):
                        wo_h = wpool.tile([P, 8 * P], bf, tag="wo", bufs=8,
                                          name=f"wo_h{h}")
                        nc.sync.dma_start(out=wo_h[:], in_=wo[dg, h])
                        wo_sbs.append(wo_h)
                    for di in range(8):
                        dcc = dg * 8 + di
                        pswo = psum.tile([P, SCW], f32, tag="psA")
                        for h in range(HPC):
                            nc.tensor.matmul(
                                pswo[:], lhsT=wo_sbs[h][:, di * P:(di + 1) * P],
                                rhs=attnT[:, h, :],
                                start=(h == 0), stop=(h == HPC - 1),
                            )
                        wcp = cpool.tile([P, SCW], bf)
                        nc.vector.tensor_copy(out=wcp[:], in_=pswo[:])
                        nc.sync.dma_start(
                            out=arin[qg][dcc * P:(dcc + 1) * P, :], in_=wcp[:]
                        )
                if stages in ("all", "noffn"):
                    nc.gpsimd.collective_compute(
                        "AllReduce", ALU.add, replica_groups=RG,
                        ins=[arin[qg][:]], outs=[arout[qg][:]],
                    )

            # ============ stage 5+6: h1, rmsnorm2, FFN, ReduceScatter =========
            for sc in (range(SCN) if stages in ("all", "ffn") else []):
                h1 = xarena.tile([P, DC, SCW], bf, tag="acts")
                ss2 = psum.tile([P, SCW], f32, tag="psA")
                for d in range(DC):
                    hld = hpool.tile([P, SCW], f32)
                    nc.sync.dma_start(
                        out=hld[:], in_=hT[d * P:(d + 1) * P, sc * SCW:(sc + 1) * SCW]
                    )
                    ars = cpool.tile([P, SCW], bf, tag="ars", bufs=2)
                    if stages == "ffn":
                        nc.vector.tensor_copy(out=ars[:], in_=hld[:])
                    else:
                        nc.sync.dma_start(
                            out=ars[:], in_=arout[sc][d * P:(d + 1) * P, :]
                        )
                    nc.vector.tensor_add(h1[:, d, :], hld[:], ars[:])
                    sq = sqpool.tile([P, SCW], bf)
                    nc.vector.tensor_mul(sq[:], h1[:, d, :], h1[:, d, :])
                    nc.tensor.matmul(
                        ss2[:], lhsT=ones_sb[:], rhs=sq[:],
                        start=(d == 0), stop=(d == DC - 1),
                    )
                rbc2 = rpool.tile([P, SCW], f32)
                nc.scalar.activation(
                    out=rbc2[:], in_=ss2[:], func=AF.Sqrt, bias=eps_t[:], scale=1.0 / D
                )
                nc.vector.reciprocal(out=rbc2[:], in_=rbc2[:])
                for d in range(DC):
                    nc.vector.tensor_mul(h1[:, d, :], h1[:, d, :], rbc2[:])

                # ---- gated FFN ----
                ffT = bpool.tile([P, FCC, SCW], bf, tag="big")
                for fc in range(FCC):
                    w0b = w01pool.tile([P, DC, P], bf, tag="w0")
                    nc.sync.dma_start(out=w0b[:], in_=w0[fc])
                    w1b = w01pool.tile([P, DC, P], bf, tag="w1")
                    nc.sync.dma_start(out=w1b[:], in_=w1[fc])
                    psg = psum.tile([P, SCW], f32, tag="psA")
                    psu = psum.tile([P, SCW], f32, tag="psA")
                    for d in range(DC):
                        nc.tensor.matmul(
                            psg[:], lhsT=w0b[:, d, :], rhs=h1[:, d, :],
                            start=(d == 0), stop=(d == DC - 1),
                        )
                        nc.tensor.matmul(
                            psu[:], lhsT=w1b[:, d, :], rhs=h1[:, d, :],
                            start=(d == 0), stop=(d == DC - 1),
                        )
                    gel = sqpool.tile([P, SCW], bf, tag="gel")
                    nc.scalar.activation(out=gel[:], in_=psg[:], func=AF.Gelu)
                    nc.vector.tensor_mul(ffT[:, fc, :], psu[:], gel[:])

                # ---- w_out partials + fold in attention partial ----
                for dg in range(8):
                    ps2 = [psum.tile([P, SCW], f32, tag="psA", name=f"ps2_{di2}") for di2 in range(4)]
                    for fc in range(FCC):
                        wob = woutpool.tile([P, 4 * P], bf)
                        nc.sync.dma_start(out=wob[:], in_=wout[dg, fc])
                        for di in range(4):
                            nc.tensor.matmul(
                                ps2[di][:], lhsT=wob[:, di * P:(di + 1) * P],
                                rhs=ffT[:, fc, :],
                                start=(fc == 0), stop=(fc == FCC - 1),
                            )
                    for di in range(4):
                        dcc = dg * 4 + di
                        rcp = cpool.tile([P, SCW], bf, tag="rcp", bufs=2)
                        if stages == "ffn":
                            nc.vector.tensor_copy(out=rcp[:], in_=ps2[di][:])
                        else:
                            arp = cpool.tile([P, SCW], bf, tag="arp", bufs=2)
                            nc.sync.dma_start(
                                out=arp[:], in_=arin[sc][dcc * P:(dcc + 1) * P, :]
                            )
                            nc.vector.tensor_add(rcp[:], ps2[di][:], arp[:])
                        nc.sync.dma_start(
                            out=rsin[sc][dcc * P:(dcc + 1) * P, :], in_=rcp[:]
                        )
                nc.gpsimd.collective_compute(
                    "ReduceScatter", ALU.add, replica_groups=RG,
                    ins=[rsin[sc][:]], outs=[rsout[sc][:]],
                )

            # ================= stage 7: final residual, output ================
            for sc in (range(SCN) if stages == "all" else [0]):
                for ol in range(WPC // P):
                    hot = hpool.tile([P, SCW], f32)
                    nc.sync.dma_start(
                        out=hot[:],
                        in_=h_own[ol * P:(ol + 1) * P, sc * SCW:(sc + 1) * SCW],
                    )
                    ot = outpool.tile([P, SCW], f32)
                    if stages in ("all", "ffn"):
                        rst = cpool.tile([P, SCW], bf, tag="rst", bufs=2)
                        nc.sync.dma_start(
                            out=rst[:], in_=rsout[sc][ol * P:(ol + 1) * P, :]
                        )
                        nc.vector.tensor_add(ot[:], hot[:], rst[:])
                    else:
                        nc.vector.tensor_copy(out=ot[:], in_=hot[:])
                    nc.sync.dma_start(
                        out=out[ol * P:(ol + 1) * P, sc * SCW:(sc + 1) * SCW],
                        in_=ot[:],
                    )

    nc.finalize()
    return nc


def _prep_in_maps(inputs):
    import ml_dtypes

    bf16 = ml_dtypes.bfloat16
    hid = np.ascontiguousarray(np.asarray(inputs["hidden_states"], np.float32)[0])
    mask = np.asarray(inputs["attention_mask"])[0]
    pbias = np.asarray(inputs["position_bias"], np.float32)[0]
    ln_a = np.asarray(inputs["ln_attn_w"], np.float32)
    ln_f = np.asarray(inputs["ln_ffn_w"], np.float32)
    wq = np.asarray(inputs["wq"], np.float32)
    wk = np.asarray(inputs["wk"], np.float32)
    wv = np.asarray(inputs["wv"], np.float32)
    wo = np.asarray(inputs["wo"], np.float32)
    w0 = np.asarray(inputs["w0"], np.float32)
    w1 = np.asarray(inputs["w1"], np.float32)
    w_out = np.asarray(inputs["w_out"], np.float32)

    hT = np.ascontiguousarray(hid.T)                          # (D, S) f32
    wq_f = (ln_a[:, None] * wq * (DH ** -0.5)).astype(bf16)
    wk_f = (ln_a[:, None] * wk).astype(bf16)
    wv_f = (ln_a[:, None] * wv).astype(bf16)
    wo_f = wo.astype(bf16)
    w0_f = (ln_f[:, None] * w0).astype(bf16)
    w1_f = (ln_f[:, None] * w1).astype(bf16)
    wout_f = w_out.astype(bf16)
    if mask.all():
        pb_m = pbias.astype(bf16)
    else:
        pb_m = np.where(mask[None], pbias, np.float32(-1e30)).astype(bf16)

    eye = np.eye(P, dtype=bf16)
    ones = np.ones((P, P), dtype=bf16)

    in_maps = []
    for c in range(NCORES):
        ws = slice(c * WPC, (c + 1) * WPC)
        fs = slice(c * FPC, (c + 1) * FPC)
        # wo: (WPC, D) -> (4 dgrp, HPC, P, 8*P): [dg,h,p,f] = wo[h*128+p, dg*1024+f]
        wo_c = wo_f[ws, :].reshape(HPC, P, 4, 8 * P).transpose(2, 0, 1, 3)
        # pb: (HPC, S, S) -> (HPC, KC qc, SCN k4, P, SCW)
        pb_c = pb_m[c * HPC:(c + 1) * HPC].reshape(HPC, KC, P, SCN, SCW)
        pb_c = pb_c.transpose(0, 1, 3, 2, 4)
        # w0/w1: (D, FPC) -> (FCC, P, DC, P): [fc,p,d,f] = w[d*128+p, fc*128+f]
        w0_c = w0_f[:, fs].reshape(DC, P, FCC, P).transpose(2, 1, 0, 3)
        w1_c = w1_f[:, fs].reshape(DC, P, FCC, P).transpose(2, 1, 0, 3)
        # wout: (FPC, D) -> (8 dg, FCC, P, 4*P): [dg,fc,p,f] = wout[fc*128+p, dg*512+f]
        wout_c = wout_f[fs, :].reshape(FCC, P, 8, 4 * P).transpose(2, 0, 1, 3)
        in_maps.append({
            "hT": hT,
            "h_own": np.ascontiguousarray(hT[ws]),
            "wq": np.ascontiguousarray(wq_f[:, ws]),
            "wk": np.ascontiguousarray(wk_f[:, ws]),
            "wv": np.ascontiguousarray(wv_f[:, ws]),
            "wo": np.ascontiguousarray(wo_c),
            "pb": np.ascontiguousarray(pb_c),
            "w0": np.ascontiguousarray(w0_c),
            "w1": np.ascontiguousarray(w1_c),
            "wout": np.ascontiguousarray(wout_c),
            "eye": eye,
            "ones": ones,
        })
    return in_maps


def get_nc(stages="all"):
    if stages not in _CACHE:
        _CACHE[stages] = _build(stages)
    return _CACHE[stages]


def kernel(**inputs):
    from concourse.bass_utils import run_bass_kernel_spmd

    nc = get_nc()
    in_maps = _prep_in_maps(inputs)
    res = run_bass_kernel_spmd(nc, in_maps, core_ids=list(range(NCORES)))
    parts = [res.results[c]["out"] for c in range(NCORES)]   # each (WPC, S)
    full_T = np.concatenate(parts, axis=0)                    # (D, S)
    out = np.ascontiguousarray(full_T.T)[None]                # (1, S, D)
    return out.astype(np.float32)

